# revision 12
# baseline (speedup 1.0000x reference)
"""Trainium2 Bass kernel for nn_MoCo_4810363372846 (retrieval_knn).

Computation (see harness reference):
    h    = relu(im_q @ W1 + b1)            [B, 2048]
    q    = (h @ W2 + b2) row-normalized    [B, 128]
    dist = mean_j sqrt((q_i-k_j) invD (q_i-k_j)^T)  over 64 sampled queue cols
    top-63 (excluding the max) rows of dist gate a masked write into
    output[:, 2:4].

Strategy:
  * Data-parallel over the B=16384 rows: 8 NeuronCores x 2048 rows each.
    Weights / invD / sampled-queue constants are replicated.
  * The dominant cost is the [2048x2048] W1 GEMM.  It runs in fp8 e4m3
    with MatmulPerfMode.DoubleRow (two 128-deep K-tiles per instruction,
    ~2x the bf16/fp32r PE rate) on 512-wide output blocks (moving free
    dim 1024, the fp8 max).  X is quantized to e4m3 and pre-transposed to
    feature-major on the host (no on-device transpose phase); W1/W2 are
    scaled by 64 into the e4m3 normal range and the 1/64 is folded into
    the activation scale.  The whole fp8 W1 (4 MB) stays resident in
    SBUF.  W2 also runs as fp8 DoubleRow; the Mahalanobis quadratic form
    quad[b,j] = r[b] + c2[j] - 2 t[j,b]  runs as a handful of tiny fp32r
    matmuls.  Each block's W2+Mahalanobis chain is software-pipelined
    into the NEXT block's W1 matmul stream (the PE queue is in-order, so
    cross-engine chain latency would otherwise stall it).  Device output:
    dist row [1, 2048] per core.
  * On host: gather the 8 dist shards and exactly recompute (fp64) the
    rows whose dist lands within BOUNDARY_WINDOW of the top-64 threshold
    (or of the max).  Measured fp8 dist error on this input distribution
    is <= 0.025 absolute; the window is 0.1.  Then stable-argsort, build
    the row mask, and apply the masked write to output columns 2/3.
"""

import functools

import numpy as np
import ml_dtypes

B, DIM_MLP, DIM, KQ, NUM = 16384, 2048, 128, 16384, 64
NCORES = 8
BL = B // NCORES  # 2048 rows per core
P = 128
K16 = DIM_MLP // P  # 16 contraction sub-tiles
NH = 512          # output block (fp8 moving max is 128x1024 -> 512 out cols)

E4 = ml_dtypes.float8_e4m3
BF16 = ml_dtypes.bfloat16

# absolute dist window around the top-64 threshold whose rows get an exact
# host-side recompute; >= 4x the measured fp8 dist error (max 0.025).
BOUNDARY_WINDOW = 0.1


@functools.lru_cache(maxsize=None)
def _build_nc(reps=1):
    import concourse.mybir as mybir
    import concourse.tile as tile
    from concourse import bacc

    f32 = mybir.dt.float32
    f32r = mybir.dt.float32r
    f8 = mybir.dt.float8e4
    AF = mybir.ActivationFunctionType
    DR = mybir.MatmulPerfMode.DoubleRow

    nc = bacc.Bacc(None, target_bir_lowering=False)

    xt = nc.declare_dram_parameter("xt", [P, K16, BL], f8, isOutput=False)
    w1 = nc.declare_dram_parameter("w1q", [P, K16, DIM_MLP], f8, isOutput=False)
    w2 = nc.declare_dram_parameter("w2q", [P, K16, DIM], f8, isOutput=False)
    b1t = nc.declare_dram_parameter("b1t", [P, K16], f32, isOutput=False)
    b2t = nc.declare_dram_parameter("b2t", [P, 1], f32, isOutput=False)
    invd = nc.declare_dram_parameter("invd", [P, P], f32, isOutput=False)
    ct = nc.declare_dram_parameter("ct", [P, NUM], f32, isOutput=False)
    c2r = nc.declare_dram_parameter("c2r", [1, NUM], f32, isOutput=False)
    dist = nc.declare_dram_parameter("dist", [1, BL], f32, isOutput=True)

    with tile.TileContext(nc) as tc:
        with (
            tc.tile_pool(name="const", bufs=1) as constp,
            tc.tile_pool(name="ht", bufs=2) as htp,
            tc.tile_pool(name="dsb", bufs=1) as dsbp,
            tc.tile_pool(name="ps_h", bufs=4, space="PSUM") as ps_h,
            tc.tile_pool(name="ps_q", bufs=1, space="PSUM") as ps_q,
            tc.tile_pool(name="ps_d", bufs=3, space="PSUM") as ps_d,
        ):
            def const_f32r(shape, value, name):
                stage = constp.tile(shape, f32, name=f"{name}_f32")
                nc.any.memset(stage, value)
                out = constp.tile(shape, f32r, name=name)
                nc.any.tensor_copy(out=out, in_=stage)
                return out

            ones_k = const_f32r([P, 1], 1.0, "ones_k")
            ones64s = const_f32r([NUM, 1], 1.0 / NUM, "ones64s")
            ones_m32 = const_f32r([1, P], 1.0, "ones_m32")

            b1s = constp.tile([P, K16], f32)
            nc.sync.dma_start(b1s, b1t[:])
            b2s = constp.tile([P, 1], f32)
            nc.sync.dma_start(b2s, b2t[:])
            invds = constp.tile([P, P], f32r)
            nc.sync.dma_start(invds, invd[:].bitcast(f32r))
            cts = constp.tile([P, NUM], f32r)
            nc.sync.dma_start(cts, ct[:].bitcast(f32r))
            # S2 = [ones; c2] stationary for the merged "-r/2 - c2/2" matmul
            s2stage = constp.tile([2, NUM], f32)
            nc.any.memset(s2stage[0:1, :], 1.0)
            nc.sync.dma_start(s2stage[1:2, :], c2r[:])
            s2c = constp.tile([2, NUM], f32r)
            nc.any.tensor_copy(out=s2c, in_=s2stage)
            # M2 moving: row0 = -r/2 (rewritten per block), row1 = -1/2 const
            m2stage = constp.tile([2, NH], f32)
            nc.any.memset(m2stage, -0.5)
            m2t = constp.tile([2, NH], f32r)
            nc.any.tensor_copy(out=m2t, in_=m2stage)
            w1s = constp.tile([P, K16, DIM_MLP], f8)
            nc.sync.dma_start(w1s, w1[:])
            w2s = constp.tile([P, K16, DIM], f8)
            nc.sync.dma_start(w2s, w2[:])
            xts = constp.tile([P, K16, BL], f8)
            # chunked so a cold first pass can start computing after chunk 0
            for c in range(BL // NH):
                nc.sync.dma_start(
                    xts[:, :, c * NH : (c + 1) * NH],
                    xt[:, :, c * NH : (c + 1) * NH],
                )
            dist_sb = constp.tile([1, BL], f32)

            def maha_chain(ht, m2):
                """W2 + normalize + Mahalanobis + dist for block m2.

                Yields after each step so the emitter can interleave the
                steps into the next block's W1 matmul stream (PE executes
                its queue in order; un-interleaved, every PE step of this
                cross-engine chain would stall the queue head).
                """
                pq = ps_q.tile([P, NH], f32, tag="pq")
                for g in range(4):
                    for k2 in (2 * g, 2 * g + 1):
                        nc.tensor.matmul(
                            pq,
                            w2s[:, 2 * k2 : 2 * k2 + 2, :],
                            ht[:, 2 * k2 : 2 * k2 + 2, :],
                            start=(k2 == 0),
                            stop=(k2 == K16 // 2 - 1),
                            perf_mode=DR,
                        )
                    yield
                qt = dsbp.tile([P, NH], f32, tag="qt")
                nc.scalar.activation(
                    qt, pq, AF.Identity, bias=b2s[:, 0:1], scale=1.0 / 64
                )
                yield
                # s = 1/||q|| per column
                sq = dsbp.tile([P, NH], f32r, tag="sq")
                nc.vector.tensor_mul(sq, qt, qt)
                yield
                pn = ps_d.tile([P, NH], f32, tag="pd")
                nc.tensor.matmul(pn[:1, :], ones_k, sq)
                yield
                nrm = dsbp.tile([1, NH], f32, tag="nrm")
                nc.scalar.activation(nrm, pn[:1, :], AF.Sqrt)
                yield
                s = dsbp.tile([1, NH], f32r, tag="s")
                with nc.allow_low_precision(reason="fp22 norm scale is ample"):
                    nc.vector.reciprocal(s, nrm)
                yield
                # qn = q * s  (s broadcast over partitions via K=1 matmul)
                pb = ps_d.tile([P, NH], f32, tag="pd")
                nc.tensor.matmul(pb, ones_m32, s)
                yield
                qn = dsbp.tile([P, NH], f32r, tag="qn")
                nc.vector.tensor_mul(qn, qt, pb)
                yield
                # r = qn^T invD qn  (per column)
                pu = ps_d.tile([P, NH], f32, tag="pd")
                nc.tensor.matmul(pu, invds, qn)
                yield
                prod = dsbp.tile([P, NH], f32r, tag="prod")
                nc.vector.tensor_mul(prod, qn, pu)
                yield
                pr = ps_d.tile([P, NH], f32, tag="pd")
                nc.tensor.matmul(pr[:1, :], ones_k, prod)
                yield
                # m2t row0 <- -r/2 (row1 is the constant -1/2)
                nc.scalar.activation(m2t[0:1, :], pr[:1, :], AF.Identity, scale=-0.5)
                yield
                # psum = t - r/2 - c2/2 = -quad/2 ;  sqrtq = sqrt(-2*psum)
                ptq = ps_d.tile([P, NH], f32, tag="pd")
                nc.tensor.matmul(
                    ptq[:NUM, :], cts, qn, start=True, stop=False
                )
                nc.tensor.matmul(
                    ptq[:NUM, :], s2c, m2t, start=False, stop=True
                )
                yield
                sqq = dsbp.tile([NUM, NH], f32r, tag="sqq")
                nc.scalar.activation(sqq, ptq[:NUM, :], AF.Sqrt, scale=-2.0)
                yield
                # dist = mean_j sqrt(quad)
                pdd = ps_d.tile([P, NH], f32, tag="pd")
                nc.tensor.matmul(pdd[:1, :], ones64s, sqq)
                yield
                o0 = m2 * NH
                nc.scalar.activation(
                    dist_sb[:, o0 : o0 + NH], pdd[:1, :], AF.Identity
                )

            pending = None
            for _rep in range(reps):
                for m2 in range(BL // NH):
                    # ---- h = relu((X @ W1q)/64 + b1), feature-major fp8 ----
                    ht = htp.tile([P, K16, NH], f8, tag="ht")
                    m0 = m2 * NH
                    for n in range(K16):
                        ph = ps_h.tile([P, NH], f32, tag="ph")
                        for k2 in range(K16 // 2):
                            nc.tensor.matmul(
                                ph,
                                w1s[:, 2 * k2 : 2 * k2 + 2, n * P : (n + 1) * P],
                                xts[:, 2 * k2 : 2 * k2 + 2, m0 : m0 + NH],
                                start=(k2 == 0),
                                stop=(k2 == K16 // 2 - 1),
                                perf_mode=DR,
                            )
                        nc.scalar.activation(
                            ht[:, n, :],
                            ph,
                            AF.Relu,
                            bias=b1s[:, n : n + 1],
                            scale=1.0 / 64,
                        )
                        if pending is not None:
                            next(pending, None)
                            if n < 4:
                                next(pending, None)
                    if pending is not None:
                        for _ in pending:
                            pass
                    pending = maha_chain(ht, m2)
            for _ in pending:
                pass

            nc.sync.dma_start(dist[:], dist_sb)

    nc.compile()
    return nc


def _host_constants(W1, b1, W2, b2, queue, invD, sample_idx):
    qs = queue[:, sample_idx].T.astype(np.float64)  # [64, 128]
    iD = invD.astype(np.float64)
    ct = (iD @ qs.T).astype(np.float32)  # [128, 64]
    c2 = np.sum((qs @ iD) * qs, axis=1).astype(np.float32)[None, :]  # [1, 64]
    b1t = np.ascontiguousarray(
        b1.astype(np.float32).reshape(K16, P).T
    )  # [128, 16]; b1t[p, no] = b1[no*128+p]
    b2t = np.ascontiguousarray(b2.astype(np.float32).reshape(P, 1))
    # fp8 weights scaled by 64 into the e4m3 normal range, k-tile-major:
    # w[p, kk, n] = e4m3(64 * W[kk*128 + p, n])
    w1q = np.ascontiguousarray(
        (W1.astype(np.float32) * 64.0)
        .astype(E4)
        .reshape(K16, P, DIM_MLP)
        .transpose(1, 0, 2)
    )
    w2q = np.ascontiguousarray(
        (W2.astype(np.float32) * 64.0)
        .astype(E4)
        .reshape(K16, P, DIM)
        .transpose(1, 0, 2)
    )
    return ct, c2, b1t, b2t, w1q, w2q


def _host_x_tiles(im_q):
    # xt[p, kk, m] = e4m3(im_q[m, kk*128 + p]); per-core slices of m.
    xq = im_q.astype(E4)  # [B, 2048]
    xtf = np.ascontiguousarray(xq.T)  # [2048, B]
    xtf = xtf.reshape(K16, P, B).transpose(1, 0, 2)  # [128, 16, B] view
    return [
        np.ascontiguousarray(xtf[:, :, i * BL : (i + 1) * BL])
        for i in range(NCORES)
    ]


def _exact_dist_rows(rows, im_q, W1, b1, W2, b2, qs64, iD64):
    X = im_q[rows].astype(np.float64)
    h = np.maximum(X @ W1.astype(np.float64) + b1.astype(np.float64), 0)
    q = h @ W2.astype(np.float64) + b2.astype(np.float64)
    q = q / np.maximum(np.linalg.norm(q, axis=1, keepdims=True), 1e-12)
    u = q @ iD64
    r = np.sum(u * q, axis=1)
    t = q @ (iD64 @ qs64.T)
    c2 = np.sum((qs64 @ iD64) * qs64, axis=1)
    quad = np.maximum(r[:, None] + c2[None, :] - 2 * t, 0)
    return np.sqrt(quad).mean(axis=1)


LAST_RESULTS = None   # for test harness introspection
LAST_DIST_DEV = None  # raw device dist before host boundary recompute


def kernel(im_q, output, sample_idx, W1, b1, W2, b2, queue, invD):
    global LAST_RESULTS, LAST_DIST_DEV
    from concourse.bass_utils import run_bass_kernel_spmd

    im_q = np.ascontiguousarray(np.asarray(im_q, dtype=np.float32))
    output = np.asarray(output, dtype=np.float32)
    sample_idx = np.asarray(sample_idx)
    W1 = np.ascontiguousarray(np.asarray(W1, dtype=np.float32))
    b1 = np.asarray(b1, dtype=np.float32)
    W2 = np.ascontiguousarray(np.asarray(W2, dtype=np.float32))
    b2 = np.asarray(b2, dtype=np.float32)
    queue = np.asarray(queue, dtype=np.float32)
    invD = np.ascontiguousarray(np.asarray(invD, dtype=np.float32))

    ct, c2, b1t, b2t, w1q, w2q = _host_constants(
        W1, b1, W2, b2, queue, invD, sample_idx
    )
    xts = _host_x_tiles(im_q)

    nc = _build_nc()
    in_maps = []
    for i in range(NCORES):
        in_maps.append(
            {
                "xt": xts[i],
                "w1q": w1q,
                "w2q": w2q,
                "b1t": b1t,
                "b2t": b2t,
                "invd": invD,
                "ct": ct,
                "c2r": c2,
            }
        )
    res = run_bass_kernel_spmd(nc, in_maps, core_ids=list(range(NCORES)))
    LAST_RESULTS = res
    dist = np.concatenate(
        [np.asarray(res.results[i]["dist"]).reshape(BL) for i in range(NCORES)]
    ).astype(np.float64)
    LAST_DIST_DEV = dist.copy()

    # exact host recompute of rows near the top-64 inclusion boundary (and the
    # max-exclusion boundary) so fp8 rounding cannot flip the selected set
    thr = np.partition(dist, B - NUM)[B - NUM]
    top1 = dist.max()
    rows = np.nonzero(
        (np.abs(dist - thr) <= BOUNDARY_WINDOW)
        | (dist >= top1 - BOUNDARY_WINDOW)
    )[0]
    if rows.size:
        qs64 = queue[:, sample_idx].T.astype(np.float64)
        iD64 = invD.astype(np.float64)
        dist[rows] = _exact_dist_rows(rows, im_q, W1, b1, W2, b2, qs64, iD64)

    order = np.argsort(dist, kind="stable")
    sel = order[-NUM:-1]
    row_mask = np.zeros(B, dtype=bool)
    row_mask[sel] = True
    cond = row_mask & ((np.abs(output[:, 2]) < 1.0) | (np.abs(output[:, 3]) < 1.0))
    out = output.copy()
    out[:, 2] = np.where(cond, np.float32(-5.0), output[:, 2])
    out[:, 3] = np.where(cond, np.float32(5.0), out[:, 3])
    return out
